# revision 40
# baseline (speedup 1.0000x reference)
"""GRU (B=64, T=512, DIN=D=512) on 8 Trainium2 NeuronCores.

Strategy
--------
Data-parallel over batch: each core owns BL = 8 batch rows, weights are
replicated (per the sharding hint).  Per core:

1. Projection phase: xg = X @ W_g + b_g for g in {z, r, h} as GEMMs with W
   stationary (bf16) and X^T streaming (bf16, so the PE streams at full
   rate - fp32 moving operands stream at half rate), written straight into
   an SBUF-resident pre-activation buffer xall[p, g, m, t*BL+b] (bf16,
   ~96KB/partition) by ScalarE Identity-with-bias ops.  ALL projection
   chunks run densely in the prologue (~90us, HAM-warm, back-to-back
   streams); interleaving them into the scan was measured strictly worse
   because the N=512 streams sit on the in-order PE queue in front of the
   scan's critical matmuls (and run cold, 427ns vs 216ns).

2. Scan phase (the sequential part): state is kept transposed,
   hT [128 partitions = d%128, KT=4 k-tiles, BL=8], so that
   - the recurrent matmuls are psum[m] += U[k,m].T @ hmT[k] (U stationary,
     state streaming, output already transposed), and
   - all elementwise work (sigmoid/tanh/blend) runs on fat [128, 32] tiles.
   The x-projection term is accumulated into PSUM by an identity matmul
   (start=True) so the activations read PSUM directly - no DVE pre-adds.
   The update gate is computed as zc = sigmoid(-zpre) = 1 - z (free affine
   scale=-1 on the ACT op), which turns the blend into
       h = (hm - zc*hm) + zc*hh
   where (hm - zc*hm) is computed off the critical path; only zc*hh and
   the final add sit between tanh and the next step's matmuls, and those
   run in k-halves so the next step's k0/k1 matmuls start after half the
   blend.  The h-gate pre-activations go to TWO separate psum tiles
   (m01/m23, m-major matmul order): the dependency tracker assigns a
   consumer's semaphore threshold from its producer tile's last writer,
   so with a single tile tanh's first half would wait for all 16 h
   matmuls; with split tiles it fires after 8 (~200ns earlier per step,
   measured 2811 -> 2605 ns/step together with the m-major order).

The mask input: reference semantics are h_t = z*(m_{t-1}*h_{t-1}) + ...,
i.e. the *shifted* mask multiplies the previous state.  For the all-ones
mask (what setup_inputs produces) this is the identity, so the fast path
skips the multiply; a general path (host-broadcast shifted mask streamed
from DRAM, one extra DVE mul per step) handles arbitrary 0/1 masks.
"""

import numpy as np
from contextlib import ExitStack

import concourse.bass as bass
import concourse.bacc as bacc
import concourse.mybir as mybir
import concourse.tile as tile
from concourse.tile import add_dep_helper
from concourse.bass_utils import run_bass_kernel_spmd

FP32 = mybir.dt.float32
BF16 = mybir.dt.bfloat16
FP8 = mybir.dt.float8e4
AF = mybir.ActivationFunctionType

import os
U_FP8 = os.environ.get("GRU_U_FP8", "0") == "1"
FP8_S = 64.0                # host-side scale on U (and on xall) when fp8

B, T, DIN, D = 64, 512, 512, 512
NCORES = 8
BL = B // NCORES            # 8 batch rows per core
KT = DIN // 128             # 4 contraction tiles
MT = D // 128               # 4 output tiles
P = 128


def build_nc(T_=T, masked=False, use_bf16=True, u_fp8=False):
    """Build the single-core SPMD program (identical on all 8 cores)."""
    tl = min(64, T_)                     # steps per chunk
    sch = T_ // tl                       # chunks
    pcw = tl * BL                        # chunk width in columns (512)
    ldt = BF16 if use_bf16 else FP32     # low-precision dtype
    udt = FP8 if u_fp8 else ldt          # recurrent-weight dtype
    # With fp8 U the host pre-scales U by FP8_S (values ~N(0, 1/sqrt(D))
    # would underflow e4m3 otherwise); xall is then also written scaled by
    # FP8_S so PSUM holds FP8_S*(x + hm@U), and the gate activations use
    # scale=1/FP8_S to undo it for free.
    xsc = FP8_S if u_fp8 else 1.0

    nc = bacc.Bacc(None, target_bir_lowering=False, debug=False)

    ldt_ = BF16 if use_bf16 else FP32
    xT = nc.dram_tensor("xT", [DIN, T_ * BL], ldt_, kind="ExternalInput")
    w_lay = {g: nc.dram_tensor(f"W{g}", [P, KT * D], ldt_, kind="ExternalInput")
             for g in "zrh"}
    u_lay = {g: nc.dram_tensor(f"U{g}", [P, KT * D], udt, kind="ExternalInput")
             for g in "zrh"}
    b4 = {g: nc.dram_tensor(f"b{g}", [P, MT], FP32, kind="ExternalInput")
          for g in "zrh"}
    eye_d = nc.dram_tensor("eye", [P, P], ldt_, kind="ExternalInput")
    mb = None
    if masked:
        mb = nc.dram_tensor("mb", [T_, P, KT * BL], FP32, kind="ExternalInput")
    hT_out = nc.dram_tensor("hT_out", [P, KT * BL], ldt_,
                            kind="ExternalOutput")

    with tile.TileContext(nc) as tc, ExitStack() as ctx:
        upool = ctx.enter_context(tc.tile_pool(name="upool", bufs=1))
        wpool = ctx.enter_context(tc.tile_pool(name="wpool", bufs=1))
        bp = ctx.enter_context(tc.tile_pool(name="bp", bufs=1))
        xap = ctx.enter_context(tc.tile_pool(name="xap", bufs=1))
        xtp = ctx.enter_context(tc.tile_pool(name="xtp", bufs=2 * KT))
        pproj = ctx.enter_context(
            tc.tile_pool(name="pproj", bufs=2, space="PSUM"))
        psc = ctx.enter_context(tc.tile_pool(name="psc", bufs=2, space="PSUM"))
        psch = ctx.enter_context(
            tc.tile_pool(name="psch", bufs=1, space="PSUM"))
        sm = ctx.enter_context(tc.tile_pool(name="sm", bufs=3))
        mbp = ctx.enter_context(tc.tile_pool(name="mbp", bufs=2))

        # DMA order: everything the first projection unit gates on (xt
        # chunk 0, W, b, eye) first; U is not consumed until the scan
        # starts ~100us in, so its transfers queue last.
        u_sb = {}
        eye_sb = upool.tile([P, P], ldt, tag="eye", name="eye")
        nc.sync.dma_start(eye_sb[:], eye_d[:])

        xt_tiles = {}

        def emit_xt_dmas(c):
            tiles = []
            for kk in range(KT):
                xt = xtp.tile([P, pcw], ldt_, tag="xt", name=f"xt{c}_{kk}")
                nc.sync.dma_start(
                    xt[:], xT[kk * P:(kk + 1) * P, c * pcw:(c + 1) * pcw])
                tiles.append(xt)
            xt_tiles[c] = tiles

        emit_xt_dmas(0)
        w_sb = {}
        b_sb = {}
        for g in "zrh":
            w_sb[g] = wpool.tile([P, KT * D], ldt_, tag=f"w{g}", name=f"w{g}")
            nc.sync.dma_start(w_sb[g][:], w_lay[g][:])
            b_sb[g] = bp.tile([P, MT], FP32, tag=f"b{g}", name=f"b{g}")
            nc.sync.dma_start(b_sb[g][:], b4[g][:])
            if u_fp8:
                nc.vector.tensor_scalar_mul(b_sb[g][:], b_sb[g][:], FP8_S)
        for g in "zrh":
            u_sb[g] = upool.tile([P, KT * D], udt, tag=f"u{g}", name=f"u{g}")
            nc.sync.dma_start(u_sb[g][:], u_lay[g][:])

        # SBUF-resident pre-activations: [p, gate, m-tile, t*BL+b]
        xall = xap.tile([P, 3, KT, T_ * BL], ldt, tag="xall", name="xall")

        gate_i = {"z": 0, "r": 1, "h": 2}

        def emit_proj_unit(c, g, m, anchor=None, anchor_act=None):
            ps = pproj.tile([P, pcw], FP32, tag="pp", name=f"pp{c}{g}{m}")
            for kk in range(KT):
                mm = nc.tensor.matmul(
                    ps[:],
                    w_sb[g][:, kk * D + m * P: kk * D + (m + 1) * P],
                    xt_tiles[c][kk][:],
                    start=(kk == 0), stop=(kk == KT - 1))
                if kk == 0 and anchor is not None:
                    # pin this hidden projection unit behind its host
                    # step's last recurrent matmul so the scheduler
                    # places it in that step's tail window instead of
                    # flooding the first scan chunk
                    add_dep_helper(mm.ins, anchor, sync=False,
                                   reason="proj placement anchor")
            ev = nc.scalar.activation(
                xall[:, gate_i[g], m, c * pcw:(c + 1) * pcw], ps[:],
                AF.Identity, bias=b_sb[g][:, m:m + 1], scale=float(xsc))
            if anchor_act is not None:
                # same for the ACT evacuation: behind the host step's
                # activations, else ScalarE's in-order queue can deadlock
                # against the pinned matmuls
                add_dep_helper(ev.ins, anchor_act, sync=False,
                               reason="proj evac placement anchor")
            return ev

        proj_units = [(c, g, m) for c in range(sch)
                      for g in "zrh" for m in range(MT)]
        # prologue: chunk 0 runs dense before the scan; chunk c+1
        # interleaves into scan chunk c for the rest
        # All projections run densely in the prologue: the PE streams them
        # back-to-back (warm HAM, ~216ns/MM) and the whole xall buffer is
        # SBUF-resident before the scan starts.  Interleaving them into the
        # scan was measured strictly worse: the N=512 streams run cold
        # (~427ns) and sit on the in-order PE queue in front of the scan's
        # critical matmuls.
        prologue_evacs = []
        for c in range(sch):
            if c > 0:
                emit_xt_dmas(c)
            for g in "zrh":
                for m in range(MT):
                    prologue_evacs.append(emit_proj_unit(c, g, m).ins)

        # MM emission order: k-halves outer, because the previous step's
        # blend produces the state in k-halves (h0 then h1) - all k0/k1
        # matmuls can start as soon as the first half lands.
        ORD_K = ([(kk, m) for kk in (0, 1) for m in range(MT)]
                 + [(kk, m) for kk in (2, 3) for m in range(MT)])

        def gate_mms(psum, g, rhs, xv, order, barrier=None, after=None,
                     extra_stop_at=None):
            # identity matmul accumulates the x-projection into PSUM first
            # (start=True, one MM covers all 4 m-regions); it has no data
            # deps beyond the projection, so PE can issue it while waiting
            # for rhs.
            skip = extra_stop_at is not None
            idmm = nc.tensor.matmul(psum[:], eye_sb[:], xv[:],
                                    start=True, stop=False,
                                    skip_group_check=skip)
            if barrier:
                # keep the scheduler from dribbling prologue work into the
                # scan: step 0 starts only after the whole prologue
                for e in barrier:
                    add_dep_helper(idmm.ins, e, sync=True,
                                   reason="prologue barrier")
            stop_mm = None
            for i, (kk, m) in enumerate(order):
                mm = nc.tensor.matmul(
                    psum[:, m],
                    u_sb[g][:, kk * D + m * P: kk * D + (m + 1) * P],
                    rhs[:, kk],
                    start=False,
                    stop=(i == len(order) - 1 or i == extra_stop_at),
                    skip_group_check=skip)
                if i == 0 and after is not None:
                    # keep this gate's matmuls from interleaving into the
                    # previous gate's block - the previous gate's PSUM
                    # completion (which gates an activation on the
                    # critical path) must not be pushed out
                    add_dep_helper(mm.ins, after, sync=False,
                                   reason="gate ordering")
                stop_mm = mm
            return idmm, stop_mm

        h_prev = sm.tile([P, KT, BL], ldt, tag="h", name="h0")
        nc.vector.memset(h_prev[:], 0.0)

        for t in range(T_):
            c = t // tl
            ti = t % tl
            if ti == 0:
                if masked:
                    mb_sb = mbp.tile([P, tl, KT * BL], FP32, tag="m",
                                     name=f"mb{c}")
                    nc.sync.dma_start(
                        mb_sb[:],
                        mb[c * tl:(c + 1) * tl].rearrange("t p x -> p t x"))

            if masked:
                hm = sm.tile([P, KT, BL], ldt, tag="hm")
                nc.vector.tensor_mul(
                    hm[:], h_prev[:],
                    mb_sb[:, ti].rearrange("p (k b) -> p k b", k=KT))
            else:
                hm = h_prev

            xv = xall[:, :, :, t * BL:(t + 1) * BL]

            bar = prologue_evacs if t == 0 else None
            # r gate
            ps_r = psc.tile([P, KT, BL], FP32, tag="pr")
            _, r_stop = gate_mms(ps_r, "r", hm, xv[:, 1], ORD_K, barrier=bar)
            r_sb = sm.tile([P, KT, BL], ldt, tag="r")
            nc.scalar.activation(r_sb[:], ps_r[:], AF.Sigmoid, scale=1.0 / xsc)
            rhm = sm.tile([P, KT, BL], ldt, tag="rhm")
            nc.vector.tensor_mul(rhm[:], r_sb[:], hm[:])

            # z gate (complement): zc = 1 - z = sigmoid(-zpre)
            ps_z = psc.tile([P, KT, BL], FP32, tag="pz")
            _, z_stop = gate_mms(ps_z, "z", hm, xv[:, 0], ORD_K, barrier=bar,
                                 after=r_stop.ins)
            zc = sm.tile([P, KT, BL], ldt, tag="zc")
            nc.scalar.activation(zc[:], ps_z[:], AF.Sigmoid, scale=-1.0 / xsc)
            # off-critical-path part of the blend: c1 = hm - zc*hm
            zchm = sm.tile([P, KT, BL], ldt, tag="zchm")
            nc.vector.tensor_mul(zchm[:], zc[:], hm[:])
            c1 = sm.tile([P, KT, BL], ldt, tag="c1")
            nc.vector.tensor_sub(c1[:], hm[:], zchm[:])

            # h candidate.  rhm is fully ready when these start, so
            # unlike r/z there is no k-half arrival constraint: emit
            # m-major into TWO separate psum tiles (m01 / m23) so the
            # dependency tracker releases tanh's first half after 8
            # matmuls instead of 16 (a split stop-flag within one tile
            # does not shorten the consumer's semaphore threshold).
            ps_h0 = psch.tile([P, 2, BL], FP32, tag="ph0")
            ps_h1 = psch.tile([P, 2, BL], FP32, tag="ph1")
            first_h = None
            h_stop = None
            for half, ps_hh in ((0, ps_h0), (1, ps_h1)):
                idmm_h = nc.tensor.matmul(
                    ps_hh[:], eye_sb[:], xv[:, 2, 2 * half:2 * half + 2],
                    start=True, stop=False)
                if bar:
                    for e in bar:
                        add_dep_helper(idmm_h.ins, e, sync=True,
                                       reason="prologue barrier")
                for m in (0, 1):
                    for kk in range(KT):
                        gm = 2 * half + m
                        mm = nc.tensor.matmul(
                            ps_hh[:, m],
                            u_sb["h"][:, kk * D + gm * P:
                                      kk * D + (gm + 1) * P],
                            rhm[:, kk],
                            start=False,
                            stop=(m == 1 and kk == KT - 1))
                        if first_h is None:
                            first_h = mm
                            add_dep_helper(mm.ins, z_stop.ins, sync=False,
                                           reason="gate ordering")
                        h_stop = mm

            # critical tail in k-halves: h = c1 + zc*hh; the next step's
            # k0/k1 matmuls only need the first half of h.  (Unsplitting
            # this tail was measured ~90ns/step worse.)
            hh = sm.tile([P, KT, BL], ldt, tag="hh")
            b2 = sm.tile([P, KT, BL], ldt, tag="b2")
            h_new = sm.tile([P, KT, BL], ldt, tag="h")
            # DVE order [b2h0, b2h1, hnh0, hnh1]: with [b2h0, hnh0, b2h1,
            # hnh1] the second mul was DVE-issue-bound behind the first
            # add (~190ns after its tanh data was ready); muls-first lets
            # it issue at data arrival, pulling hnh1 (which gates the
            # next step's k2/k3 matmuls) ~64ns earlier.
            for hf, ps_hh in ((0, ps_h0), (1, ps_h1)):
                sl = slice(2 * hf, 2 * hf + 2)
                nc.scalar.activation(hh[:, sl], ps_hh[:], AF.Tanh,
                                     scale=1.0 / xsc)
                nc.vector.tensor_mul(b2[:, sl], zc[:, sl], hh[:, sl])
            for hf in range(2):
                sl = slice(2 * hf, 2 * hf + 2)
                nc.vector.tensor_add(h_new[:, sl], c1[:, sl], b2[:, sl])
            h_prev = h_new

        nc.sync.dma_start(
            hT_out[:].rearrange("p (k b) -> p k b", k=KT), h_prev[:])

    nc.compile()
    return nc


_NC_CACHE = {}


def _get_nc(masked, use_bf16=True):
    key = (masked, use_bf16, U_FP8)
    if key not in _NC_CACHE:
        _NC_CACHE[key] = build_nc(T, masked=masked, use_bf16=use_bf16,
                                  u_fp8=U_FP8)
    return _NC_CACHE[key]


def _w_layout(w):
    # [DIN, D] -> [128, KT*D] with lay[p, kk*D + j] = w[kk*128 + p, j]
    return np.ascontiguousarray(
        w.reshape(KT, P, D).transpose(1, 0, 2).reshape(P, KT * D), dtype=np.float32)


def _b_layout(b):
    return np.ascontiguousarray(b.reshape(MT, P).T, dtype=np.float32)


def make_in_maps(X, W_z, U_z, b_z, W_r, U_r, b_r, W_h, U_h, b_h, mask,
                 masked):
    import ml_dtypes
    bf16 = ml_dtypes.bfloat16
    X = np.asarray(X, dtype=np.float32)
    shared = {"eye": np.eye(P, dtype=np.float32).astype(bf16)}
    for g, w, u, b in (("z", W_z, U_z, b_z), ("r", W_r, U_r, b_r),
                       ("h", W_h, U_h, b_h)):
        shared[f"W{g}"] = _w_layout(np.asarray(w, dtype=np.float32)).astype(bf16)
        ug = _w_layout(np.asarray(u, dtype=np.float32))
        if U_FP8:
            shared[f"U{g}"] = (ug * FP8_S).astype(
                mybir.dt.np(mybir.dt.float8e4))
        else:
            shared[f"U{g}"] = ug.astype(bf16)
        shared[f"b{g}"] = _b_layout(np.asarray(b, dtype=np.float32))

    in_maps = []
    for c in range(NCORES):
        bsl = slice(c * BL, (c + 1) * BL)
        m = dict(shared)
        m["xT"] = np.ascontiguousarray(
            X[bsl].transpose(2, 1, 0).reshape(DIN, T * BL)).astype(bf16)
        if masked:
            msh = np.zeros((T, BL), dtype=np.float32)
            msh[1:] = np.asarray(mask)[bsl, :T - 1].T.astype(np.float32)
            m["mb"] = np.ascontiguousarray(
                np.tile(msh[:, None, :], (1, P, KT)))
        in_maps.append(m)
    return in_maps


def kernel(X, W_z, U_z, b_z, W_r, U_r, b_r, W_h, U_h, b_h, mask):
    mask = np.asarray(mask)
    masked = not bool(np.all(mask[:, :T - 1] == 1))
    nc = _get_nc(masked)
    in_maps = make_in_maps(X, W_z, U_z, b_z, W_r, U_r, b_r, W_h, U_h, b_h,
                           mask, masked)
    res = run_bass_kernel_spmd(nc, in_maps, core_ids=list(range(NCORES)))
    out = np.empty((B, D), dtype=np.float32)
    for c in range(NCORES):
        arr = res.results[c]["hT_out"].astype(np.float32)
        out[c * BL:(c + 1) * BL] = (
            arr.reshape(P, KT, BL).transpose(2, 1, 0).reshape(BL, D))
    return out

